# revision 11
# baseline (speedup 1.0000x reference)
"""CharMIRNN Trainium2 kernel.

Strategy (8 NeuronCores, data-parallel over batch B=64 -> 8 rows/core):
  - Embedding gather + input projection fused: A/C tables are one-hot
    matmuls against pre-scaled tables derived from W1 = emb @ U_w.T:
      A = alpha*Ux + beta1 = onehot @ (alpha*W1 + beta1)
      C = beta2*Ux + bias  = onehot @ (beta2*W1 + bias)
  - Recurrence per step: Vh = h @ V_w.T done as matmul with stationary
    lhsT = h.T (tiny weight load) streaming V_w.T (fp16, 1 col/cycle).
    m = Vh*A_t + C_t on DVE, tanh on ACT, h.T recovered via DMA-xbar
    transposes (16-row padded), accumulated into an SBUF-resident hs.T.
  - Decode: logits.T-free layout; lhsT = hs.T tiles, rhs = dec_w.T fp16,
    bias via a K=1 ones-row matmul. Output DMA'd straight into
    [B,L,V]-layout DRAM.
"""
import numpy as np

import concourse.bass as bass
import concourse.tile as tile
import concourse.mybir as mybir
from concourse import bacc
from concourse.bass_utils import run_bass_kernel_spmd
from concourse.masks import make_identity
from contextlib import ExitStack

B, L, V, E, H = 64, 512, 256, 512, 1024
NCORES = 8
BL = B // NCORES          # batch rows per core
f32 = mybir.dt.float32
f16 = mybir.dt.float16
f32r = mybir.dt.float32r
i32 = mybir.dt.int32
mult = mybir.AluOpType.mult
addop = mybir.AluOpType.add
iseq = mybir.AluOpType.is_equal
Tanh = mybir.ActivationFunctionType.Tanh


def _emit(nc, Ls, DEBUG=False):
    T = BL * Ls           # tokens per core
    TC = T // 128         # token chunks
    x_d = nc.declare_dram_parameter("x", [BL, Ls], i32, isOutput=False)
    emb_d = nc.declare_dram_parameter("emb", [V, E], f32, isOutput=False)
    uw_d = nc.declare_dram_parameter("U_w", [H, E], f32, isOutput=False)
    vw_d = nc.declare_dram_parameter("V_w", [H, H], f32, isOutput=False)
    al_d = nc.declare_dram_parameter("alpha", [1, H], f32, isOutput=False)
    b1_d = nc.declare_dram_parameter("beta1", [1, H], f32, isOutput=False)
    b2_d = nc.declare_dram_parameter("beta2", [1, H], f32, isOutput=False)
    bi_d = nc.declare_dram_parameter("bias", [1, H], f32, isOutput=False)
    dw_d = nc.declare_dram_parameter("dec_w", [V, H], f32, isOutput=False)
    db_d = nc.declare_dram_parameter("dec_b", [1, V], f32, isOutput=False)
    lg_d = nc.declare_dram_parameter("logits", [BL, Ls, V], f32, isOutput=True)
    hl_d = nc.declare_dram_parameter("h_last", [BL, H], f32, isOutput=True)
    dbg = {}
    if DEBUG:
        dbg["w1"] = nc.declare_dram_parameter("dbg_w1", [2, 128, H], f32, isOutput=True)
        dbg["oh"] = nc.declare_dram_parameter("dbg_oh", [128, 2 * BL * Ls], f16, isOutput=True)
        dbg["a"] = nc.declare_dram_parameter("dbg_a", [BL * Ls, H], f16, isOutput=True)
        dbg["c"] = nc.declare_dram_parameter("dbg_c", [BL * Ls, H], f16, isOutput=True)
        dbg["hsT"] = nc.declare_dram_parameter("dbg_hsT", [128, 8 * BL * Ls], f16, isOutput=True)
        dbg["vwT"] = nc.declare_dram_parameter("dbg_vwT", [128, 8 * H], f16, isOutput=True)
        dbg["wa"] = nc.declare_dram_parameter("dbg_wa", [2, 128, H], f16, isOutput=True)
        dbg["wc"] = nc.declare_dram_parameter("dbg_wc", [2, 128, H], f16, isOutput=True)

    with tile.TileContext(nc) as tc, ExitStack() as ctx:
        persist = ctx.enter_context(tc.tile_pool(name="persist", bufs=1))
        dram = ctx.enter_context(tc.tile_pool(name="dram", bufs=1, space="DRAM"))

        ident = persist.tile([128, 128], f32)
        make_identity(nc, ident[:])
        vwT = persist.tile([128, 8 * H], f16)       # V_w.T  [j*128+p, i] at col 1024*j+i
        hsT = persist.tile([128, 8 * T], f16)       # hs.T chunk j at col T*j + (l*BL+b)
        dwT = persist.tile([128, 8 * V], f16)       # dec_w.T chunk k at col V*k + v
        ones_r = persist.tile([1, 128], f16)
        nc.vector.memset(ones_r[:], 1.0)
        decb = persist.tile([1, V], f16)
        hn = [persist.tile([16, H], f16, tag=f"hn{i}", name=f"hn{i}") for i in range(2)]
        for t_ in hn:
            nc.vector.memset(t_[:, :], 0.0)
        hl_sb = persist.tile([BL, H], f32)
        a_dram = dram.tile([T, H], f16)
        c_dram = dram.tile([T, H], f16)

        # ---------- prologue ----------
        with (
            tc.tile_pool(name="pro", bufs=2) as pro,
            tc.tile_pool(name="pro1", bufs=1) as pro1,
            tc.tile_pool(name="pps", bufs=4, space="PSUM") as pps,
        ):
            # dec_b row (fp16)
            dbf = pro1.tile([1, V], f32)
            nc.sync.dma_start(dbf[:], db_d[:])
            nc.vector.tensor_copy(decb[:], dbf[:])

            # V_w.T fp16
            for ic in range(8):
                stg = pro.tile([128, H], f32, tag="stage")
                nc.sync.dma_start(stg[:], vw_d[128 * ic:128 * (ic + 1), :])
                for jc in range(8):
                    pst = pps.tile([128, 128], f32, tag="tp")
                    nc.tensor.transpose(pst[:], stg[:, 128 * jc:128 * (jc + 1)], ident[:])
                    nc.vector.tensor_copy(vwT[:, H * jc + 128 * ic: H * jc + 128 * (ic + 1)], pst[:])

            # dec_w.T fp16
            for vc in range(2):
                stg = pro.tile([128, H], f32, tag="stage")
                nc.sync.dma_start(stg[:], dw_d[128 * vc:128 * (vc + 1), :])
                for kc in range(8):
                    pst = pps.tile([128, 128], f32, tag="tp")
                    nc.tensor.transpose(pst[:], stg[:, 128 * kc:128 * (kc + 1)], ident[:])
                    nc.vector.tensor_copy(dwT[:, V * kc + 128 * vc: V * kc + 128 * (vc + 1)], pst[:])

            # emb.T (f32r) [512, 256]: chunk kc at col 256*kc+v
            embT = pro1.tile([128, 4 * V], f32r)
            for vc in range(2):
                stg = pro.tile([128, H], f32, tag="stage")
                nc.sync.dma_start(stg[:, :E], emb_d[128 * vc:128 * (vc + 1), :])
                for kc in range(4):
                    pst = pps.tile([128, 128], f32, tag="tp")
                    nc.tensor.transpose(pst[:], stg[:, 128 * kc:128 * (kc + 1)], ident[:])
                    nc.vector.tensor_copy(embT[:, V * kc + 128 * vc: V * kc + 128 * (vc + 1)], pst[:])

            # U_w.T (f32r) [512, 1024]: chunk kc at col 1024*kc+h
            uwT = pro1.tile([128, 4 * H], f32r)
            for hc in range(8):
                stg = pro.tile([128, H], f32, tag="stage")
                nc.sync.dma_start(stg[:, :E], uw_d[128 * hc:128 * (hc + 1), :])
                for kc in range(4):
                    pst = pps.tile([128, 128], f32, tag="tp")
                    nc.tensor.transpose(pst[:], stg[:, 128 * kc:128 * (kc + 1)], ident[:])
                    nc.vector.tensor_copy(uwT[:, H * kc + 128 * hc: H * kc + 128 * (hc + 1)], pst[:])

            # W1 = emb @ U_w.T  -> [256, 1024] f32 (2 chunks on partitions)
            w1 = [pro1.tile([128, H], f32, tag=f"w1_{m}", name=f"w1_{m}") for m in range(2)]
            for mv in range(2):
                for nh in range(2):
                    ps = pps.tile([128, 512], f32, tag="mm")
                    for kc in range(4):
                        nc.tensor.matmul(
                            ps[:],
                            embT[:, V * kc + 128 * mv: V * kc + 128 * (mv + 1)],
                            uwT[:, H * kc + 512 * nh: H * kc + 512 * (nh + 1)],
                            start=(kc == 0), stop=(kc == 3),
                        )
                    nc.vector.tensor_copy(w1[mv][:, 512 * nh:512 * (nh + 1)], ps[:])

            # WA = alpha*W1 + beta1 ; WC = beta2*W1 + bias   (f32r)
            vbc = []
            for r, d in enumerate([al_d, b1_d, b2_d, bi_d]):
                t_ = pro1.tile([128, H], f16, name=f"vbc{r}")
                nc.gpsimd.dma_start(t_[:], d[:].to_broadcast((128, H)))
                vbc.append(t_)
            wa = [pro1.tile([128, H], f16, tag=f"wa_{m}", name=f"wa_{m}") for m in range(2)]
            wc = [pro1.tile([128, H], f16, tag=f"wc_{m}", name=f"wc_{m}") for m in range(2)]
            tmp = pro1.tile([128, H], f32)
            for mv in range(2):
                nc.vector.tensor_tensor(tmp[:], w1[mv][:], vbc[0][:], op=mult)
                nc.vector.tensor_tensor(wa[mv][:], tmp[:], vbc[1][:], op=addop)
                nc.vector.tensor_tensor(tmp[:], w1[mv][:], vbc[2][:], op=mult)
                nc.vector.tensor_tensor(wc[mv][:], tmp[:], vbc[3][:], op=addop)

            # one-hot.T (f32r): chunk vc at col T*vc + t ; t = b*Ls + l
            xi = pro1.tile([128, T], i32)
            nc.sync.dma_start(xi[:], x_d[:].rearrange("b l -> (b l)").unsqueeze(0).partition_broadcast(128))
            iot = pro1.tile([128, 2], i32)
            nc.gpsimd.iota(iot[:], pattern=[[128, 2]], base=0, channel_multiplier=1)
            iotf = pro1.tile([128, 2], f32)
            nc.vector.tensor_copy(iotf[:], iot[:])
            oh = pro1.tile([128, 2 * T], f16)
            for vc in range(2):
                nc.vector.tensor_scalar(
                    oh[:, T * vc:T * (vc + 1)], xi[:], iotf[:, vc:vc + 1], None, op0=iseq)

            # A/C tables: one-hot matmuls, evict fp16 -> DRAM
            for tcc in range(TC):
                for wmat, dst in ((wa, a_dram), (wc, c_dram)):
                    for nh in range(2):
                        ps = pps.tile([128, 512], f32, tag="mm")
                        for vc in range(2):
                            nc.tensor.matmul(
                                ps[:],
                                oh[:, T * vc + 128 * tcc: T * vc + 128 * (tcc + 1)],
                                wmat[vc][:, 512 * nh:512 * (nh + 1)],
                                start=(vc == 0), stop=(vc == 1),
                            )
                        ev = pro.tile([128, 512], f16, tag="ac_ev")
                        if nh == 0:
                            nc.vector.tensor_copy(ev[:], ps[:])
                        else:
                            nc.scalar.activation(ev[:], ps[:], mybir.ActivationFunctionType.Copy)
                        nc.sync.dma_start(
                            dst[128 * tcc:128 * (tcc + 1), 512 * nh:512 * (nh + 1)], ev[:])
            if DEBUG:
                for mv in range(2):
                    nc.sync.dma_start(dbg["w1"][mv], w1[mv][:])
                    nc.sync.dma_start(dbg["wa"][mv], wa[mv][:])
                    nc.sync.dma_start(dbg["wc"][mv], wc[mv][:])
                nc.sync.dma_start(dbg["oh"][:], oh[:])

        # ---------- recurrence ----------
        a_re = a_dram[:].rearrange("(b l) h -> b l h", l=Ls)
        c_re = c_dram[:].rearrange("(b l) h -> b l h", l=Ls)
        with (
            tc.tile_pool(name="ht", bufs=3) as htp,
            tc.tile_pool(name="acl", bufs=4) as aclp,
            tc.tile_pool(name="wk", bufs=4) as wk,
            tc.tile_pool(name="rps", bufs=4, space="PSUM") as rps,
        ):
            hT_prev = htp.tile([128, 128], f16, tag="hT")
            nc.vector.memset(hT_prev[:], 0.0)
            for l in range(Ls):
                hT_cur = htp.tile([128, 128], f16, tag="hT")
                a_l = aclp.tile([BL, H], f16, tag="a_l")
                c_l = aclp.tile([BL, H], f16, tag="c_l")
                nc.sync.dma_start(a_l[:], a_re[:, l, :])
                nc.sync.dma_start(c_l[:], c_re[:, l, :])
                hcur = hn[l % 2]
                for nh in range(2):
                    ps = rps.tile([BL, 512], f32, tag="vh")
                    for j in range(8):
                        nc.tensor.matmul(
                            ps[:],
                            hT_prev[:, 16 * j:16 * j + BL],
                            vwT[:, H * j + 512 * nh: H * j + 512 * (nh + 1)],
                            start=(j == 0), stop=(j == 7),
                        )
                    for q in range(2):
                        c0 = 512 * nh + 256 * q
                        m0 = wk.tile([BL, 256], f32, tag="m0")
                        nc.vector.tensor_tensor(m0[:], ps[:, 256 * q:256 * (q + 1)], a_l[:, c0:c0 + 256], op=mult)
                        m1 = wk.tile([BL, 256], f32, tag="m1")
                        nc.vector.tensor_tensor(m1[:], m0[:], c_l[:, c0:c0 + 256], op=addop)
                        nc.scalar.activation(hcur[0:BL, c0:c0 + 256], m1[:], Tanh)
                        if l == Ls - 1:
                            nc.scalar.activation(hl_sb[:, c0:c0 + 256], m1[:], Tanh)
                        for t2 in range(2):
                            j = 2 * (2 * nh + q) + t2
                            nc.sync.dma_start_transpose(
                                hT_cur[:, 16 * j:16 * (j + 1)], hcur[0:16, 128 * j:128 * (j + 1)])
                nc.vector.tensor_copy(
                    hsT[:].rearrange("p (j t) -> p j t", t=T)[:, :, BL * l:BL * (l + 1)],
                    hT_cur[:].rearrange("p (j c) -> p j c", c=16)[:, :, 0:BL],
                )
                hT_prev = hT_cur
            nc.sync.dma_start(hl_d[:], hl_sb[:])

        # ---------- decode ----------
        lg_re = lg_d[:].rearrange("b l v -> l b v")
        LB = Ls * BL
        with (
            tc.tile_pool(name="dec", bufs=4) as dec,
            tc.tile_pool(name="dps", bufs=4, space="PSUM") as dps,
        ):
            for tcc in range(TC):
                ps = dps.tile([128, V], f32, tag="dmm")
                for j in range(8):
                    nc.tensor.matmul(
                        ps[:],
                        hsT[:, T * j + 128 * tcc: T * j + 128 * (tcc + 1)],
                        dwT[:, V * j:V * (j + 1)],
                        start=(j == 0), stop=False,
                    )
                nc.tensor.matmul(ps[:], ones_r[:], decb[:], start=False, stop=True)
                ev = dec.tile([128, V], f32, tag="dec_ev")
                if tcc % 2 == 0:
                    nc.vector.tensor_copy(ev[:], ps[:])
                else:
                    nc.scalar.activation(ev[:], ps[:], mybir.ActivationFunctionType.Copy)
                nl = 128 // BL
                nc.sync.dma_start(lg_re[nl * tcc:nl * (tcc + 1)], ev[:])
        if DEBUG:
            nc.sync.dma_start(dbg["hsT"][:], hsT[:])
            nc.sync.dma_start(dbg["vwT"][:], vwT[:])
            nc.sync.dma_start(dbg["a"][:], a_dram[:])
            nc.sync.dma_start(dbg["c"][:], c_dram[:])
    nc.compile()
    return nc


_CACHE = {}


def _get_nc(Ls, DEBUG=False):
    key = (Ls, DEBUG)
    if key not in _CACHE:
        nc = bacc.Bacc("TRN2", target_bir_lowering=False, debug=False,
                       num_devices=NCORES)
        _CACHE[key] = _emit(nc, Ls, DEBUG=DEBUG)
    return _CACHE[key]


def kernel(x, emb, U_w, V_w, alpha, beta1, beta2, bias, dec_w, dec_b,
           _trace=False, _debug=False):
    Ls = x.shape[1]
    nc = _get_nc(Ls, DEBUG=_debug)
    x = np.asarray(x)
    shared = {
        "emb": np.asarray(emb, np.float32),
        "U_w": np.asarray(U_w, np.float32),
        "V_w": np.asarray(V_w, np.float32),
        "alpha": np.asarray(alpha, np.float32).reshape(1, H),
        "beta1": np.asarray(beta1, np.float32).reshape(1, H),
        "beta2": np.asarray(beta2, np.float32).reshape(1, H),
        "bias": np.asarray(bias, np.float32).reshape(1, H),
        "dec_w": np.asarray(dec_w, np.float32),
        "dec_b": np.asarray(dec_b, np.float32).reshape(1, V),
    }
    in_maps = [
        {"x": np.ascontiguousarray(x[c * BL:(c + 1) * BL]).astype(np.int32), **shared}
        for c in range(NCORES)
    ]
    res = run_bass_kernel_spmd(nc, in_maps, list(range(NCORES)), trace=_trace)
    logits = np.concatenate([res.results[c]["logits"] for c in range(NCORES)], axis=0)
    h_last = np.concatenate([res.results[c]["h_last"] for c in range(NCORES)], axis=0)
    kernel._last_results = res
    return logits, h_last[None, :, :]


# revision 14
# speedup vs baseline: 2.6372x; 2.6372x over previous
"""CharMIRNN Trainium2 kernel.

Strategy (8 NeuronCores, data-parallel over batch B=64 -> 8 rows/core):
  - Embedding gather + input projection fused: A/C tables are one-hot
    matmuls against pre-scaled tables derived from W1 = emb @ U_w.T:
      A = alpha*Ux + beta1 = onehot @ (alpha*W1 + beta1)
      C = beta2*Ux + bias  = onehot @ (beta2*W1 + bias)
  - Recurrence per step: Vh = h @ V_w.T done as matmul with stationary
    lhsT = h.T (tiny weight load) streaming V_w.T (fp16, 1 col/cycle).
    m = Vh*A_t + C_t on DVE, tanh on ACT, h.T recovered via DMA-xbar
    transposes (16-row padded), accumulated into an SBUF-resident hs.T.
  - Decode: logits.T-free layout; lhsT = hs.T tiles, rhs = dec_w.T fp16,
    bias via a K=1 ones-row matmul. Output DMA'd straight into
    [B,L,V]-layout DRAM.
"""
import numpy as np

import concourse.bass as bass
import concourse.tile as tile
import concourse.mybir as mybir
from concourse import bacc
from concourse.bass_utils import run_bass_kernel_spmd
from concourse.masks import make_identity
from contextlib import ExitStack

B, L, V, E, H = 64, 512, 256, 512, 1024
NCORES = 8
BL = B // NCORES          # batch rows per core
f32 = mybir.dt.float32
f16 = mybir.dt.float16
f32r = mybir.dt.float32r
i32 = mybir.dt.int32
mult = mybir.AluOpType.mult
addop = mybir.AluOpType.add
iseq = mybir.AluOpType.is_equal
Tanh = mybir.ActivationFunctionType.Tanh


def _emit(nc, Ls, DEBUG=False):
    T = BL * Ls           # tokens per core
    TC = T // 128         # token chunks
    x_d = nc.declare_dram_parameter("x", [BL, Ls], i32, isOutput=False)
    emb_d = nc.declare_dram_parameter("emb", [V, E], f32, isOutput=False)
    uw_d = nc.declare_dram_parameter("U_w", [H, E], f32, isOutput=False)
    vw_d = nc.declare_dram_parameter("V_w", [H, H], f32, isOutput=False)
    al_d = nc.declare_dram_parameter("alpha", [1, H], f32, isOutput=False)
    b1_d = nc.declare_dram_parameter("beta1", [1, H], f32, isOutput=False)
    b2_d = nc.declare_dram_parameter("beta2", [1, H], f32, isOutput=False)
    bi_d = nc.declare_dram_parameter("bias", [1, H], f32, isOutput=False)
    dw_d = nc.declare_dram_parameter("dec_w", [V, H], f32, isOutput=False)
    db_d = nc.declare_dram_parameter("dec_b", [1, V], f32, isOutput=False)
    lg_d = nc.declare_dram_parameter("logits", [BL, Ls, V], f32, isOutput=True)
    hl_d = nc.declare_dram_parameter("h_last", [BL, H], f32, isOutput=True)
    dbg = {}
    if DEBUG:
        dbg["w1"] = nc.declare_dram_parameter("dbg_w1", [2, 128, H], f32, isOutput=True)
        dbg["oh"] = nc.declare_dram_parameter("dbg_oh", [128, 2 * BL * Ls], f16, isOutput=True)
        dbg["a"] = nc.declare_dram_parameter("dbg_a", [BL * Ls, H], f16, isOutput=True)
        dbg["c"] = nc.declare_dram_parameter("dbg_c", [BL * Ls, H], f16, isOutput=True)
        dbg["hsT"] = nc.declare_dram_parameter("dbg_hsT", [128, 8 * BL * Ls], f16, isOutput=True)
        dbg["vwT"] = nc.declare_dram_parameter("dbg_vwT", [128, 8 * H], f16, isOutput=True)
        dbg["wa"] = nc.declare_dram_parameter("dbg_wa", [2, 128, H], f16, isOutput=True)
        dbg["wc"] = nc.declare_dram_parameter("dbg_wc", [2, 128, H], f16, isOutput=True)

    with tile.TileContext(nc) as tc, ExitStack() as ctx:
        persist = ctx.enter_context(tc.tile_pool(name="persist", bufs=1))
        dram = ctx.enter_context(tc.tile_pool(name="dram", bufs=1, space="DRAM"))

        ident = persist.tile([128, 128], f32)
        make_identity(nc, ident[:])
        identh = persist.tile([128, 128], f16)
        make_identity(nc, identh[:])
        vwT = persist.tile([128, 8 * H], f16)       # V_w.T  [j*128+p, i] at col 1024*j+i
        hsT = persist.tile([128, 8 * T], f16)       # hs.T chunk j at col T*j + (l*BL+b)
        dwT = persist.tile([128, 8 * V], f16)       # dec_w.T chunk k at col V*k + v
        ones_r = persist.tile([1, 128], f16)
        nc.vector.memset(ones_r[:], 1.0)
        decb = persist.tile([1, V], f16)
        hn = [persist.tile([8, H], f16, tag=f"hn{i}", name=f"hn{i}") for i in range(2)]
        zT = persist.tile([128, 64], f16)
        nc.vector.memset(zT[:], 0.0)
        hl_sb = persist.tile([BL, H], f32)
        a_dram = dram.tile([T, H], f16)
        c_dram = dram.tile([T, H], f16)

        # ---------- prologue ----------
        with (
            tc.tile_pool(name="pro", bufs=2) as pro,
            tc.tile_pool(name="pro1", bufs=1) as pro1,
            tc.tile_pool(name="pps", bufs=4, space="PSUM") as pps,
        ):
            # dec_b row (fp16)
            dbf = pro1.tile([1, V], f32)
            nc.sync.dma_start(dbf[:], db_d[:])
            nc.vector.tensor_copy(decb[:], dbf[:])

            # V_w.T fp16
            for ic in range(8):
                stg = pro.tile([128, H], f32, tag="stage")
                nc.sync.dma_start(stg[:], vw_d[128 * ic:128 * (ic + 1), :])
                for jc in range(8):
                    pst = pps.tile([128, 128], f32, tag="tp")
                    nc.tensor.transpose(pst[:], stg[:, 128 * jc:128 * (jc + 1)], ident[:])
                    nc.vector.tensor_copy(vwT[:, H * jc + 128 * ic: H * jc + 128 * (ic + 1)], pst[:])

            # dec_w.T fp16
            for vc in range(2):
                stg = pro.tile([128, H], f32, tag="stage")
                nc.sync.dma_start(stg[:], dw_d[128 * vc:128 * (vc + 1), :])
                for kc in range(8):
                    pst = pps.tile([128, 128], f32, tag="tp")
                    nc.tensor.transpose(pst[:], stg[:, 128 * kc:128 * (kc + 1)], ident[:])
                    nc.vector.tensor_copy(dwT[:, V * kc + 128 * vc: V * kc + 128 * (vc + 1)], pst[:])

            # emb.T (f32r) [512, 256]: chunk kc at col 256*kc+v
            embT = pro1.tile([128, 4 * V], f32r)
            for vc in range(2):
                stg = pro.tile([128, H], f32, tag="stage")
                nc.sync.dma_start(stg[:, :E], emb_d[128 * vc:128 * (vc + 1), :])
                for kc in range(4):
                    pst = pps.tile([128, 128], f32, tag="tp")
                    nc.tensor.transpose(pst[:], stg[:, 128 * kc:128 * (kc + 1)], ident[:])
                    nc.vector.tensor_copy(embT[:, V * kc + 128 * vc: V * kc + 128 * (vc + 1)], pst[:])

            # U_w.T (f32r) [512, 1024]: chunk kc at col 1024*kc+h
            uwT = pro1.tile([128, 4 * H], f32r)
            for hc in range(8):
                stg = pro.tile([128, H], f32, tag="stage")
                nc.sync.dma_start(stg[:, :E], uw_d[128 * hc:128 * (hc + 1), :])
                for kc in range(4):
                    pst = pps.tile([128, 128], f32, tag="tp")
                    nc.tensor.transpose(pst[:], stg[:, 128 * kc:128 * (kc + 1)], ident[:])
                    nc.vector.tensor_copy(uwT[:, H * kc + 128 * hc: H * kc + 128 * (hc + 1)], pst[:])

            # W1 = emb @ U_w.T  -> [256, 1024] f32 (2 chunks on partitions)
            w1 = [pro1.tile([128, H], f32, tag=f"w1_{m}", name=f"w1_{m}") for m in range(2)]
            for mv in range(2):
                for nh in range(2):
                    ps = pps.tile([128, 512], f32, tag="mm")
                    for kc in range(4):
                        nc.tensor.matmul(
                            ps[:],
                            embT[:, V * kc + 128 * mv: V * kc + 128 * (mv + 1)],
                            uwT[:, H * kc + 512 * nh: H * kc + 512 * (nh + 1)],
                            start=(kc == 0), stop=(kc == 3),
                        )
                    nc.vector.tensor_copy(w1[mv][:, 512 * nh:512 * (nh + 1)], ps[:])

            # WA = alpha*W1 + beta1 ; WC = beta2*W1 + bias   (f32r)
            vbc = []
            for r, d in enumerate([al_d, b1_d, b2_d, bi_d]):
                t_ = pro1.tile([128, H], f16, name=f"vbc{r}")
                nc.gpsimd.dma_start(t_[:], d[:].to_broadcast((128, H)))
                vbc.append(t_)
            wa = [pro1.tile([128, H], f16, tag=f"wa_{m}", name=f"wa_{m}") for m in range(2)]
            wc = [pro1.tile([128, H], f16, tag=f"wc_{m}", name=f"wc_{m}") for m in range(2)]
            tmp = pro1.tile([128, H], f32)
            for mv in range(2):
                nc.vector.tensor_tensor(tmp[:], w1[mv][:], vbc[0][:], op=mult)
                nc.vector.tensor_tensor(wa[mv][:], tmp[:], vbc[1][:], op=addop)
                nc.vector.tensor_tensor(tmp[:], w1[mv][:], vbc[2][:], op=mult)
                nc.vector.tensor_tensor(wc[mv][:], tmp[:], vbc[3][:], op=addop)

            # one-hot.T (f32r): chunk vc at col T*vc + t ; t = b*Ls + l
            xi = pro1.tile([128, T], i32)
            nc.sync.dma_start(xi[:], x_d[:].rearrange("b l -> (b l)").unsqueeze(0).partition_broadcast(128))
            iot = pro1.tile([128, 2], i32)
            nc.gpsimd.iota(iot[:], pattern=[[128, 2]], base=0, channel_multiplier=1)
            iotf = pro1.tile([128, 2], f32)
            nc.vector.tensor_copy(iotf[:], iot[:])
            oh = pro1.tile([128, 2 * T], f16)
            for vc in range(2):
                nc.vector.tensor_scalar(
                    oh[:, T * vc:T * (vc + 1)], xi[:], iotf[:, vc:vc + 1], None, op0=iseq)

            # A/C tables: one-hot matmuls, evict fp16 -> DRAM
            for tcc in range(TC):
                for wmat, dst in ((wa, a_dram), (wc, c_dram)):
                    for nh in range(2):
                        ps = pps.tile([128, 512], f32, tag="mm")
                        for vc in range(2):
                            nc.tensor.matmul(
                                ps[:],
                                oh[:, T * vc + 128 * tcc: T * vc + 128 * (tcc + 1)],
                                wmat[vc][:, 512 * nh:512 * (nh + 1)],
                                start=(vc == 0), stop=(vc == 1),
                            )
                        ev = pro.tile([128, 512], f16, tag="ac_ev")
                        nc.vector.tensor_copy(ev[:], ps[:])
                        nc.sync.dma_start(
                            dst[128 * tcc:128 * (tcc + 1), 512 * nh:512 * (nh + 1)], ev[:])
            if DEBUG:
                for mv in range(2):
                    nc.sync.dma_start(dbg["w1"][mv], w1[mv][:])
                    nc.sync.dma_start(dbg["wa"][mv], wa[mv][:])
                    nc.sync.dma_start(dbg["wc"][mv], wc[mv][:])
                nc.sync.dma_start(dbg["oh"][:], oh[:])

        # ---------- recurrence ----------
        a_re = a_dram[:].rearrange("(b l) h -> b l h", l=Ls)
        c_re = c_dram[:].rearrange("(b l) h -> b l h", l=Ls)
        with (
            tc.tile_pool(name="acl", bufs=4) as aclp,
            tc.tile_pool(name="wk", bufs=4) as wk,
            tc.tile_pool(name="rps", bufs=4, space="PSUM") as rps,
            tc.tile_pool(name="tps", bufs=3, space="PSUM") as tps,
        ):
            for l in range(Ls):
                pass
                a_l = aclp.tile([BL, H], f16, tag="a_l")
                c_l = aclp.tile([BL, H], f16, tag="c_l")
                nc.sync.dma_start(a_l[:], a_re[:, l, :])
                nc.sync.dma_start(c_l[:], c_re[:, l, :])
                hcur = hn[l % 2]
                ps_t = tps.tile([128, 64], f16, tag="hT")
                for nh in range(2):
                    ps = rps.tile([BL, 512], f32, tag="vh")
                    for j in range(8):
                        lhsT = (zT[:, 8 * j:8 * (j + 1)] if l == 0 else
                                hsT[:, T * j + BL * (l - 1): T * j + BL * l])
                        nc.tensor.matmul(
                            ps[:],
                            lhsT,
                            vwT[:, H * j + 512 * nh: H * j + 512 * (nh + 1)],
                            start=(j == 0), stop=(j == 7),
                        )
                    for q in range(2):
                        c0 = 512 * nh + 256 * q
                        m0 = wk.tile([BL, 256], f32, tag="m0")
                        nc.vector.tensor_tensor(m0[:], ps[:, 256 * q:256 * (q + 1)], a_l[:, c0:c0 + 256], op=mult)
                        m1 = wk.tile([BL, 256], f32, tag="m1")
                        nc.vector.tensor_tensor(m1[:], m0[:], c_l[:, c0:c0 + 256], op=addop)
                        nc.scalar.activation(hcur[:, c0:c0 + 256], m1[:], Tanh)
                        if l == Ls - 1:
                            nc.scalar.activation(hl_sb[:, c0:c0 + 256], m1[:], Tanh)
                        for t2 in range(2):
                            j = 2 * (2 * nh + q) + t2
                            nc.tensor.transpose(
                                ps_t[:, 8 * j:8 * (j + 1)],
                                hcur[:, 128 * j:128 * (j + 1)],
                                identh[0:BL, 0:BL],
                            )
                nc.scalar.activation(
                    hsT[:].rearrange("p (j t) -> p j t", t=T)[:, :, BL * l:BL * (l + 1)],
                    ps_t[:].rearrange("p (j b) -> p j b", b=BL),
                    mybir.ActivationFunctionType.Copy,
                )
            nc.sync.dma_start(hl_d[:], hl_sb[:])

        # ---------- decode ----------
        lg_re = lg_d[:].rearrange("b l v -> l b v")
        with (
            tc.tile_pool(name="dec", bufs=4) as dec,
            tc.tile_pool(name="dps", bufs=4, space="PSUM") as dps,
        ):
            for tcc in range(TC):
                ps = dps.tile([128, V], f32, tag="dmm")
                for j in range(8):
                    nc.tensor.matmul(
                        ps[:],
                        hsT[:, T * j + 128 * tcc: T * j + 128 * (tcc + 1)],
                        dwT[:, V * j:V * (j + 1)],
                        start=(j == 0), stop=False,
                    )
                nc.tensor.matmul(ps[:], ones_r[:], decb[:], start=False, stop=True)
                ev = dec.tile([128, V], f32, tag="dec_ev")
                nc.vector.tensor_copy(ev[:], ps[:])
                nl = 128 // BL
                nc.sync.dma_start(lg_re[nl * tcc:nl * (tcc + 1)], ev[:])
        if DEBUG:
            nc.sync.dma_start(dbg["hsT"][:], hsT[:])
            nc.sync.dma_start(dbg["vwT"][:], vwT[:])
            nc.sync.dma_start(dbg["a"][:], a_dram[:])
            nc.sync.dma_start(dbg["c"][:], c_dram[:])
    nc.compile()
    return nc


_CACHE = {}


def _get_nc(Ls, DEBUG=False):
    key = (Ls, DEBUG)
    if key not in _CACHE:
        nc = bacc.Bacc("TRN2", target_bir_lowering=False, debug=False,
                       num_devices=NCORES)
        _CACHE[key] = _emit(nc, Ls, DEBUG=DEBUG)
    return _CACHE[key]


def kernel(x, emb, U_w, V_w, alpha, beta1, beta2, bias, dec_w, dec_b,
           _trace=False, _debug=False):
    Ls = x.shape[1]
    nc = _get_nc(Ls, DEBUG=_debug)
    x = np.asarray(x)
    shared = {
        "emb": np.asarray(emb, np.float32),
        "U_w": np.asarray(U_w, np.float32),
        "V_w": np.asarray(V_w, np.float32),
        "alpha": np.asarray(alpha, np.float32).reshape(1, H),
        "beta1": np.asarray(beta1, np.float32).reshape(1, H),
        "beta2": np.asarray(beta2, np.float32).reshape(1, H),
        "bias": np.asarray(bias, np.float32).reshape(1, H),
        "dec_w": np.asarray(dec_w, np.float32),
        "dec_b": np.asarray(dec_b, np.float32).reshape(1, V),
    }
    in_maps = [
        {"x": np.ascontiguousarray(x[c * BL:(c + 1) * BL]).astype(np.int32), **shared}
        for c in range(NCORES)
    ]
    res = run_bass_kernel_spmd(nc, in_maps, list(range(NCORES)), trace=_trace)
    logits = np.concatenate([res.results[c]["logits"] for c in range(NCORES)], axis=0)
    h_last = np.concatenate([res.results[c]["h_last"] for c in range(NCORES)], axis=0)
    kernel._last_results = res
    return logits, h_last[None, :, :]


# revision 16
# speedup vs baseline: 2.9673x; 1.1252x over previous
"""CharMIRNN Trainium2 kernel.

Strategy (8 NeuronCores, data-parallel over batch B=64 -> 8 rows/core):
  - Embedding gather + input projection fused: A/C tables are one-hot
    matmuls against pre-scaled tables derived from W1 = emb @ U_w.T:
      A = alpha*Ux + beta1 = onehot @ (alpha*W1 + beta1)
      C = beta2*Ux + bias  = onehot @ (beta2*W1 + bias)
  - Recurrence per step: Vh = h @ V_w.T done as matmul with stationary
    lhsT = h.T (tiny weight load) streaming V_w.T (fp16, 1 col/cycle).
    m = Vh*A_t + C_t on DVE, tanh on ACT, h.T recovered via DMA-xbar
    transposes (16-row padded), accumulated into an SBUF-resident hs.T.
  - Decode: logits.T-free layout; lhsT = hs.T tiles, rhs = dec_w.T fp16,
    bias via a K=1 ones-row matmul. Output DMA'd straight into
    [B,L,V]-layout DRAM.
"""
import numpy as np

import concourse.bass as bass
import concourse.tile as tile
import concourse.mybir as mybir
from concourse import bacc
from concourse.bass_utils import run_bass_kernel_spmd
from concourse.masks import make_identity
from contextlib import ExitStack

B, L, V, E, H = 64, 512, 256, 512, 1024
NCORES = 8
BL = B // NCORES          # batch rows per core
f32 = mybir.dt.float32
f16 = mybir.dt.float16
f32r = mybir.dt.float32r
i32 = mybir.dt.int32
mult = mybir.AluOpType.mult
addop = mybir.AluOpType.add
iseq = mybir.AluOpType.is_equal
Tanh = mybir.ActivationFunctionType.Tanh


def _emit(nc, Ls, DEBUG=False):
    T = BL * Ls           # tokens per core
    TC = T // 128         # token chunks
    x_d = nc.declare_dram_parameter("x", [BL, Ls], i32, isOutput=False)
    emb_d = nc.declare_dram_parameter("emb", [V, E], f32, isOutput=False)
    uw_d = nc.declare_dram_parameter("U_w", [H, E], f32, isOutput=False)
    vw_d = nc.declare_dram_parameter("V_w", [H, H], f32, isOutput=False)
    al_d = nc.declare_dram_parameter("alpha", [1, H], f32, isOutput=False)
    b1_d = nc.declare_dram_parameter("beta1", [1, H], f32, isOutput=False)
    b2_d = nc.declare_dram_parameter("beta2", [1, H], f32, isOutput=False)
    bi_d = nc.declare_dram_parameter("bias", [1, H], f32, isOutput=False)
    dw_d = nc.declare_dram_parameter("dec_w", [V, H], f32, isOutput=False)
    db_d = nc.declare_dram_parameter("dec_b", [1, V], f32, isOutput=False)
    lg_d = nc.declare_dram_parameter("logits", [BL, Ls, V], f32, isOutput=True)
    hl_d = nc.declare_dram_parameter("h_last", [BL, H], f32, isOutput=True)
    dbg = {}
    if DEBUG:
        dbg["w1"] = nc.declare_dram_parameter("dbg_w1", [2, 128, H], f32, isOutput=True)
        dbg["oh"] = nc.declare_dram_parameter("dbg_oh", [128, 2 * BL * Ls], f16, isOutput=True)
        dbg["a"] = nc.declare_dram_parameter("dbg_a", [BL * Ls, H], f16, isOutput=True)
        dbg["c"] = nc.declare_dram_parameter("dbg_c", [BL * Ls, H], f16, isOutput=True)
        dbg["hsT"] = nc.declare_dram_parameter("dbg_hsT", [128, 8 * BL * Ls], f16, isOutput=True)
        dbg["vwT"] = nc.declare_dram_parameter("dbg_vwT", [128, 8 * H], f16, isOutput=True)
        dbg["wa"] = nc.declare_dram_parameter("dbg_wa", [2, 128, H], f16, isOutput=True)
        dbg["wc"] = nc.declare_dram_parameter("dbg_wc", [2, 128, H], f16, isOutput=True)

    with tile.TileContext(nc) as tc, ExitStack() as ctx:
        persist = ctx.enter_context(tc.tile_pool(name="persist", bufs=1))
        dram = ctx.enter_context(tc.tile_pool(name="dram", bufs=1, space="DRAM"))

        ident = persist.tile([128, 128], f32)
        make_identity(nc, ident[:])
        identh = persist.tile([128, 128], f16)
        make_identity(nc, identh[:])
        vwT = persist.tile([128, 8 * H], f16)       # V_w.T  [j*128+p, i] at col 1024*j+i
        hsT = persist.tile([128, 8 * T], f16)       # hs.T chunk j at col T*j + (l*BL+b)
        dwT = persist.tile([128, 8 * V], f16)       # dec_w.T chunk k at col V*k + v
        ones_r = persist.tile([1, 128], f16)
        nc.vector.memset(ones_r[:], 1.0)
        decb = persist.tile([1, V], f16)
        hn = [persist.tile([8, H], f16, tag=f"hn{i}", name=f"hn{i}") for i in range(2)]
        zT = persist.tile([128, 64], f16)
        nc.vector.memset(zT[:], 0.0)
        hl_sb = persist.tile([BL, H], f32)
        a_dram = dram.tile([T, H], f16)
        c_dram = dram.tile([T, H], f16)

        # ---------- prologue ----------
        with (
            tc.tile_pool(name="pro", bufs=2) as pro,
            tc.tile_pool(name="pro1", bufs=1) as pro1,
            tc.tile_pool(name="pps", bufs=4, space="PSUM") as pps,
        ):
            # dec_b row (fp16)
            dbf = pro1.tile([1, V], f32)
            nc.sync.dma_start(dbf[:], db_d[:])
            nc.vector.tensor_copy(decb[:], dbf[:])

            # V_w.T fp16
            for ic in range(8):
                stg = pro.tile([128, H], f32, tag="stage")
                nc.sync.dma_start(stg[:], vw_d[128 * ic:128 * (ic + 1), :])
                for jc in range(8):
                    pst = pps.tile([128, 128], f32, tag="tp")
                    nc.tensor.transpose(pst[:], stg[:, 128 * jc:128 * (jc + 1)], ident[:])
                    nc.vector.tensor_copy(vwT[:, H * jc + 128 * ic: H * jc + 128 * (ic + 1)], pst[:])

            # dec_w.T fp16
            for vc in range(2):
                stg = pro.tile([128, H], f32, tag="stage")
                nc.sync.dma_start(stg[:], dw_d[128 * vc:128 * (vc + 1), :])
                for kc in range(8):
                    pst = pps.tile([128, 128], f32, tag="tp")
                    nc.tensor.transpose(pst[:], stg[:, 128 * kc:128 * (kc + 1)], ident[:])
                    nc.vector.tensor_copy(dwT[:, V * kc + 128 * vc: V * kc + 128 * (vc + 1)], pst[:])

            # emb.T (f32r) [512, 256]: chunk kc at col 256*kc+v
            embT = pro1.tile([128, 4 * V], f32r)
            for vc in range(2):
                stg = pro.tile([128, H], f32, tag="stage")
                nc.sync.dma_start(stg[:, :E], emb_d[128 * vc:128 * (vc + 1), :])
                for kc in range(4):
                    pst = pps.tile([128, 128], f32, tag="tp")
                    nc.tensor.transpose(pst[:], stg[:, 128 * kc:128 * (kc + 1)], ident[:])
                    nc.vector.tensor_copy(embT[:, V * kc + 128 * vc: V * kc + 128 * (vc + 1)], pst[:])

            # U_w.T (f32r) [512, 1024]: chunk kc at col 1024*kc+h
            uwT = pro1.tile([128, 4 * H], f32r)
            for hc in range(8):
                stg = pro.tile([128, H], f32, tag="stage")
                nc.sync.dma_start(stg[:, :E], uw_d[128 * hc:128 * (hc + 1), :])
                for kc in range(4):
                    pst = pps.tile([128, 128], f32, tag="tp")
                    nc.tensor.transpose(pst[:], stg[:, 128 * kc:128 * (kc + 1)], ident[:])
                    nc.vector.tensor_copy(uwT[:, H * kc + 128 * hc: H * kc + 128 * (hc + 1)], pst[:])

            # W1 = emb @ U_w.T  -> [256, 1024] f32 (2 chunks on partitions)
            w1 = [pro1.tile([128, H], f32, tag=f"w1_{m}", name=f"w1_{m}") for m in range(2)]
            for mv in range(2):
                for nh in range(2):
                    ps = pps.tile([128, 512], f32, tag="mm")
                    for kc in range(4):
                        nc.tensor.matmul(
                            ps[:],
                            embT[:, V * kc + 128 * mv: V * kc + 128 * (mv + 1)],
                            uwT[:, H * kc + 512 * nh: H * kc + 512 * (nh + 1)],
                            start=(kc == 0), stop=(kc == 3),
                        )
                    nc.vector.tensor_copy(w1[mv][:, 512 * nh:512 * (nh + 1)], ps[:])

            # WA = alpha*W1 + beta1 ; WC = beta2*W1 + bias   (f32r)
            vbc = []
            for r, d in enumerate([al_d, b1_d, b2_d, bi_d]):
                t_ = pro1.tile([128, H], f16, name=f"vbc{r}")
                nc.gpsimd.dma_start(t_[:], d[:].to_broadcast((128, H)))
                vbc.append(t_)
            wa = [pro1.tile([128, H], f16, tag=f"wa_{m}", name=f"wa_{m}") for m in range(2)]
            wc = [pro1.tile([128, H], f16, tag=f"wc_{m}", name=f"wc_{m}") for m in range(2)]
            tmp = pro1.tile([128, H], f32)
            for mv in range(2):
                nc.vector.tensor_tensor(tmp[:], w1[mv][:], vbc[0][:], op=mult)
                nc.vector.tensor_tensor(wa[mv][:], tmp[:], vbc[1][:], op=addop)
                nc.vector.tensor_tensor(tmp[:], w1[mv][:], vbc[2][:], op=mult)
                nc.vector.tensor_tensor(wc[mv][:], tmp[:], vbc[3][:], op=addop)

            # one-hot.T (f32r): chunk vc at col T*vc + t ; t = b*Ls + l
            xi = pro1.tile([128, T], i32)
            nc.sync.dma_start(xi[:], x_d[:].rearrange("b l -> (b l)").unsqueeze(0).partition_broadcast(128))
            iot = pro1.tile([128, 2], i32)
            nc.gpsimd.iota(iot[:], pattern=[[128, 2]], base=0, channel_multiplier=1)
            iotf = pro1.tile([128, 2], f32)
            nc.vector.tensor_copy(iotf[:], iot[:])
            oh = pro1.tile([128, 2 * T], f16)
            for vc in range(2):
                nc.vector.tensor_scalar(
                    oh[:, T * vc:T * (vc + 1)], xi[:], iotf[:, vc:vc + 1], None, op0=iseq)

            # A/C tables: one-hot matmuls, evict fp16 -> DRAM
            for tcc in range(TC):
                for wmat, dst in ((wa, a_dram), (wc, c_dram)):
                    for nh in range(2):
                        ps = pps.tile([128, 512], f32, tag="mm")
                        for vc in range(2):
                            nc.tensor.matmul(
                                ps[:],
                                oh[:, T * vc + 128 * tcc: T * vc + 128 * (tcc + 1)],
                                wmat[vc][:, 512 * nh:512 * (nh + 1)],
                                start=(vc == 0), stop=(vc == 1),
                            )
                        ev = pro.tile([128, 512], f16, tag="ac_ev")
                        nc.vector.tensor_copy(ev[:], ps[:])
                        nc.sync.dma_start(
                            dst[128 * tcc:128 * (tcc + 1), 512 * nh:512 * (nh + 1)], ev[:])
            if DEBUG:
                for mv in range(2):
                    nc.sync.dma_start(dbg["w1"][mv], w1[mv][:])
                    nc.sync.dma_start(dbg["wa"][mv], wa[mv][:])
                    nc.sync.dma_start(dbg["wc"][mv], wc[mv][:])
                nc.sync.dma_start(dbg["oh"][:], oh[:])

        # ---------- recurrence ----------
        a_re = a_dram[:].rearrange("(b l) h -> b l h", l=Ls)
        c_re = c_dram[:].rearrange("(b l) h -> b l h", l=Ls)
        with (
            tc.tile_pool(name="acl", bufs=6) as aclp,
            tc.tile_pool(name="wk", bufs=4) as wk,
            tc.tile_pool(name="rps", bufs=4, space="PSUM") as rps,
            tc.tile_pool(name="tps", bufs=3, space="PSUM") as tps,
        ):
            for l in range(Ls):
                pass
                a_l = aclp.tile([BL, H], f16, tag="a_l")
                c_l = aclp.tile([BL, H], f16, tag="c_l")
                nc.sync.dma_start(a_l[:], a_re[:, l, :])
                nc.sync.dma_start(c_l[:], c_re[:, l, :])
                hcur = hn[l % 2]
                ps_t = tps.tile([128, 64], f16, tag="hT")
                for nh in range(2):
                    ps = rps.tile([BL, 512], f32, tag="vh")
                    for j in range(8):
                        lhsT = (zT[:, 8 * j:8 * (j + 1)] if l == 0 else
                                hsT[:, T * j + BL * (l - 1): T * j + BL * l])
                        nc.tensor.matmul(
                            ps[:],
                            lhsT,
                            vwT[:, H * j + 512 * nh: H * j + 512 * (nh + 1)],
                            start=(j == 0), stop=(j == 7),
                        )
                    for q in range(2):
                        c0 = 512 * nh + 256 * q
                        m0 = wk.tile([BL, 256], f32, tag="m0")
                        nc.vector.tensor_tensor(m0[:], ps[:, 256 * q:256 * (q + 1)], a_l[:, c0:c0 + 256], op=mult)
                        m1 = wk.tile([BL, 256], f32, tag="m1")
                        nc.vector.tensor_tensor(m1[:], m0[:], c_l[:, c0:c0 + 256], op=addop)
                        nc.scalar.activation(hcur[:, c0:c0 + 256], m1[:], Tanh)
                        if l == Ls - 1:
                            nc.scalar.activation(hl_sb[:, c0:c0 + 256], m1[:], Tanh)
                        for t2 in range(2):
                            j = 2 * (2 * nh + q) + t2
                            nc.tensor.transpose(
                                ps_t[:, 8 * j:8 * (j + 1)],
                                hcur[:, 128 * j:128 * (j + 1)],
                                identh[0:BL, 0:BL],
                            )
                        qq = 2 * nh + q
                        nc.scalar.activation(
                            hsT[:].rearrange("p (j t) -> p j t", t=T)[:, 2 * qq:2 * qq + 2, BL * l:BL * (l + 1)],
                            ps_t[:, 16 * qq:16 * (qq + 1)].rearrange("p (j b) -> p j b", b=BL),
                            mybir.ActivationFunctionType.Copy,
                        )

            nc.sync.dma_start(hl_d[:], hl_sb[:])

        # ---------- decode ----------
        lg_re = lg_d[:].rearrange("b l v -> l b v")
        with (
            tc.tile_pool(name="dec", bufs=4) as dec,
            tc.tile_pool(name="dps", bufs=4, space="PSUM") as dps,
        ):
            for tcc in range(TC):
                ps = dps.tile([128, V], f32, tag="dmm")
                for j in range(8):
                    nc.tensor.matmul(
                        ps[:],
                        hsT[:, T * j + 128 * tcc: T * j + 128 * (tcc + 1)],
                        dwT[:, V * j:V * (j + 1)],
                        start=(j == 0), stop=False,
                    )
                nc.tensor.matmul(ps[:], ones_r[:], decb[:], start=False, stop=True)
                ev = dec.tile([128, V], f32, tag="dec_ev")
                nc.vector.tensor_copy(ev[:], ps[:])
                nl = 128 // BL
                nc.sync.dma_start(lg_re[nl * tcc:nl * (tcc + 1)], ev[:])
        if DEBUG:
            nc.sync.dma_start(dbg["hsT"][:], hsT[:])
            nc.sync.dma_start(dbg["vwT"][:], vwT[:])
            nc.sync.dma_start(dbg["a"][:], a_dram[:])
            nc.sync.dma_start(dbg["c"][:], c_dram[:])
    nc.compile()
    return nc


_CACHE = {}


def _get_nc(Ls, DEBUG=False):
    key = (Ls, DEBUG)
    if key not in _CACHE:
        nc = bacc.Bacc("TRN2", target_bir_lowering=False, debug=False,
                       num_devices=NCORES)
        _CACHE[key] = _emit(nc, Ls, DEBUG=DEBUG)
    return _CACHE[key]


def kernel(x, emb, U_w, V_w, alpha, beta1, beta2, bias, dec_w, dec_b,
           _trace=False, _debug=False):
    Ls = x.shape[1]
    nc = _get_nc(Ls, DEBUG=_debug)
    x = np.asarray(x)
    shared = {
        "emb": np.asarray(emb, np.float32),
        "U_w": np.asarray(U_w, np.float32),
        "V_w": np.asarray(V_w, np.float32),
        "alpha": np.asarray(alpha, np.float32).reshape(1, H),
        "beta1": np.asarray(beta1, np.float32).reshape(1, H),
        "beta2": np.asarray(beta2, np.float32).reshape(1, H),
        "bias": np.asarray(bias, np.float32).reshape(1, H),
        "dec_w": np.asarray(dec_w, np.float32),
        "dec_b": np.asarray(dec_b, np.float32).reshape(1, V),
    }
    in_maps = [
        {"x": np.ascontiguousarray(x[c * BL:(c + 1) * BL]).astype(np.int32), **shared}
        for c in range(NCORES)
    ]
    res = run_bass_kernel_spmd(nc, in_maps, list(range(NCORES)), trace=_trace)
    logits = np.concatenate([res.results[c]["logits"] for c in range(NCORES)], axis=0)
    h_last = np.concatenate([res.results[c]["h_last"] for c in range(NCORES)], axis=0)
    kernel._last_results = res
    return logits, h_last[None, :, :]
